# revision 29
# baseline (speedup 1.0000x reference)
"""Multi-head attention (B=2, S=2048, D=1024, H=16) on 8 Trainium2 NeuronCores.

Sharding: tensor-parallel over heads x data-parallel over batch.
  core c -> batch g = c // 4, head group r = c % 4 (global heads 4r..4r+3).
Each core computes qkv projections for its 4 heads (w_qkv column slices),
attention for those heads over the full sequence of its batch, then an
AllToAll inside each 4-core group converts the head-sharded attention
output into a sequence-sharded one, so every core runs the full output
projection for its own 512 sequence rows. Host-side work is only
slicing/transposition of inputs and concatenation of outputs.

Device algorithm (per core):
  qT, kT   [256, 2048]  (partition = head*64+hd, free = seq)
  V        [2048, 256]  (partition = seq, free = head*64+hd)
  per (sq-chunk 1024, head pair):
    per sk-tile (128 keys):
      scoresT[sk, sq] = kT.T @ qT   (two K=64 matmuls row-packed in PE)
      pT = exp(0.125 * scoresT)     (ACT, PSUM -> SBUF)
      rowsum_acc += pT              (DVE)
      outT += V.T-slice @ pT        (PSUM accumulate, col-packed pairs)
    rowsum broadcast = ones[128,128].T @ rowsum_acc  (reduce over sk partitions)
    outT *= 1/rowsum  (DVE reciprocal + mul) -> attn_outT [256, 2048]
  AllToAll( [4 s-chunks x 256 head-rows, 512] ) -> [1024 head-rows, 512 own s]
  out[512, 1024] = attn_outT.T @ w_proj + b_proj
"""

import os
import sys

import numpy as np

try:
    import ml_dtypes
    BF16_NP = ml_dtypes.bfloat16
except ImportError:  # pragma: no cover
    BF16_NP = None

for _p in ("/opt/trn_rl_repo",):
    if os.path.isdir(_p) and _p not in sys.path:
        sys.path.append(_p)

import concourse.bass as bass  # noqa: E402
import concourse.mybir as mybir  # noqa: E402
import concourse.tile as tile  # noqa: E402
from concourse import bacc  # noqa: E402
from concourse.bass_utils import run_bass_kernel_spmd  # noqa: E402

B, S, D = 2, 2048, 1024
H, HD = 16, 64
N_CORES = 8
GROUP = 4  # cores per batch group
LH = H // GROUP  # local heads per core = 4
LHD = LH * HD  # 256 local head dims
S_OWN = S // GROUP  # 512 sequence rows owned for the projection
FP32 = mybir.dt.float32
FP32R = mybir.dt.float32r
BF16 = mybir.dt.bfloat16

SQ_CHUNK = 1024  # query-range processed per inner attention pass
N_SQ = S // SQ_CHUNK  # 2
N_SK = S // 128  # 16 key tiles
N_KT = D // 128  # 8 contraction tiles for the projections

_compiled = None
_ONES = np.ones((128, 128), dtype=np.float32)
_ONES16 = None  # set lazily (needs ml_dtypes)


def _build():
    nc = bacc.Bacc(
        "TRN2", target_bir_lowering=False, debug=False, num_devices=N_CORES
    )

    xT_d = nc.dram_tensor("xT", [D, S], BF16, kind="ExternalInput")
    wq_d = nc.dram_tensor("wq", [D, LHD], BF16, kind="ExternalInput")
    wk_d = nc.dram_tensor("wk", [D, LHD], BF16, kind="ExternalInput")
    wv_d = nc.dram_tensor("wv", [D, LHD], BF16, kind="ExternalInput")
    wp_d = nc.dram_tensor("wp", [D, D], BF16, kind="ExternalInput")
    bq_d = nc.dram_tensor("bq", [LHD, 1], FP32, kind="ExternalInput")
    bk_d = nc.dram_tensor("bk", [LHD, 1], FP32, kind="ExternalInput")
    bv_d = nc.dram_tensor("bv", [128, LHD], FP32, kind="ExternalInput")
    bp_d = nc.dram_tensor("bp", [128, D], FP32, kind="ExternalInput")
    ones_d = nc.dram_tensor("ones", [128, 128], FP32R, kind="ExternalInput")
    ones16_d = nc.dram_tensor("ones16", [128, 8], BF16, kind="ExternalInput")
    out_d = nc.dram_tensor("out", [S_OWN, D], FP32, kind="ExternalOutput")

    # Two AllGathers (one per head pair) inside each 4-core batch group;
    # the first runs while the second pair's attention still computes.
    ag_in = [[nc.dram_tensor(f"ag_in{p}_{c}", [128, SQ_CHUNK], BF16)
              for c in range(N_SQ)] for p in range(2)]
    ag_out = [nc.dram_tensor(f"ag_out{p}", [N_SQ * GROUP * 128, SQ_CHUNK], BF16)
              for p in range(2)]
    groups = [[0, 1, 2, 3], [4, 5, 6, 7]]

    with tile.TileContext(nc) as tc:
        import contextlib

        with contextlib.ExitStack() as stk:
            # ---- long-lived pools -------------------------------------
            qk_pool = stk.enter_context(tc.tile_pool(name="qk", bufs=1))
            v_pool = stk.enter_context(tc.tile_pool(name="v", bufs=1))
            ao_pool = stk.enter_context(tc.tile_pool(name="ao", bufs=1))
            const_pool = stk.enter_context(tc.tile_pool(name="const", bufs=1))
            wp_pool = stk.enter_context(tc.tile_pool(name="wp", bufs=1))

            qT = [qk_pool.tile([128, S], BF16, name=f"qT{j}", tag=f"qT{j}") for j in range(2)]
            kT = [qk_pool.tile([128, S], BF16, name=f"kT{j}", tag=f"kT{j}") for j in range(2)]
            vp = [v_pool.tile([128, LHD], BF16, name=f"v{m}", tag=f"v{m}") for m in range(16)]
            aoT = [ao_pool.tile([128, S], BF16, name=f"ao{p}", tag=f"ao{p}") for p in range(2)]

            ones_t = const_pool.tile([128, 128], FP32R, tag="ones")
            nc.sync.dma_start(ones_t[:], ones_d.ap())
            bq_t = [const_pool.tile([128, 1], FP32, name=f"bq{j}", tag=f"bq{j}")
                    for j in range(2)]
            bk_t = [const_pool.tile([128, 1], FP32, name=f"bk{j}", tag=f"bk{j}")
                    for j in range(2)]
            bv_t = const_pool.tile([128, LHD], FP32, tag="bv")
            bp_t = const_pool.tile([128, D], FP32, tag="bp")
            for j in range(2):
                jsl = slice(j * 128, (j + 1) * 128)
                nc.sync.dma_start(bq_t[j][:], bq_d.ap()[jsl, :])
                nc.sync.dma_start(bk_t[j][:], bk_d.ap()[jsl, :])
            nc.sync.dma_start(bv_t[:], bv_d.ap())
            nc.sync.dma_start(bp_t[:], bp_d.ap())

            wp_t = [wp_pool.tile([128, D], BF16, name=f"wp{k}", tag=f"wp{k}") for k in range(N_KT)]

            # ---- phase A: qkv projections -----------------------------
            with (
                tc.tile_pool(name="x", bufs=1) as x_pool,
                tc.tile_pool(name="w", bufs=1) as w_pool,
                tc.tile_pool(name="psA", bufs=1, space="PSUM") as psA,
            ):
                x_t = [x_pool.tile([128, S], BF16, name=f"x{k}", tag=f"x{k}") for k in range(N_KT)]
                wq_t = [w_pool.tile([128, LHD], BF16, name=f"wq{k}", tag=f"wq{k}") for k in range(N_KT)]
                wk_t = [w_pool.tile([128, LHD], BF16, name=f"wk{k}", tag=f"wk{k}") for k in range(N_KT)]
                wv_t = [w_pool.tile([128, LHD], BF16, name=f"wv{k}", tag=f"wv{k}") for k in range(N_KT)]
                # interleave x/w loads k-major and fan out across four DMA
                # queues so the first contraction tiles land quickly
                dma_engs = [nc.sync, nc.scalar]
                for k in range(N_KT):
                    sl = slice(k * 128, (k + 1) * 128)
                    eng = dma_engs[k % 2]
                    eng.dma_start(x_t[k][:], xT_d.ap()[sl, :])
                    eng2 = dma_engs[(k + 1) % 2]
                    eng2.dma_start(wq_t[k][:], wq_d.ap()[sl, :])
                    eng2.dma_start(wk_t[k][:], wk_d.ap()[sl, :])
                    eng2.dma_start(wv_t[k][:], wv_d.ap()[sl, :])

                # qT / kT / V in PSUM waves, contraction k outermost so the
                # PE follows the xT tiles as they stream in from HBM
                for j in range(2):
                    ps_q = [psA.tile([128, 512], FP32, name=f"psq{j}{sc}", tag=f"psA{sc}") for sc in range(4)]
                    ps_k = [psA.tile([128, 512], FP32, name=f"psk{j}{sc}", tag=f"psA{sc+4}") for sc in range(4)]
                    for k in range(N_KT):
                        for sc in range(4):
                            ssl = slice(sc * 512, (sc + 1) * 512)
                            nc.tensor.matmul(
                                ps_q[sc][:],
                                wq_t[k][:, j * 128 : (j + 1) * 128],
                                x_t[k][:, ssl],
                                start=(k == 0), stop=(k == N_KT - 1),
                            )
                            nc.tensor.matmul(
                                ps_k[sc][:],
                                wk_t[k][:, j * 128 : (j + 1) * 128],
                                x_t[k][:, ssl],
                                start=(k == 0), stop=(k == N_KT - 1),
                            )
                    for sc in range(4):
                        ssl = slice(sc * 512, (sc + 1) * 512)
                        nc.vector.tensor_scalar(
                            qT[j][:, ssl], ps_q[sc][:], bq_t[j][:], None,
                            mybir.AluOpType.add,
                        )
                        nc.vector.tensor_scalar(
                            kT[j][:, ssl], ps_k[sc][:], bk_t[j][:], None,
                            mybir.AluOpType.add,
                        )
                # V: [s-tile 128, 256] = x.T @ wv, two waves of 8 m-tiles
                for wave in range(2):
                    ps_v = [psA.tile([128, LHD], FP32, name=f"psv{wave}{i}", tag=f"psA{i}") for i in range(8)]
                    for k in range(N_KT):
                        for i in range(8):
                            m = wave * 8 + i
                            nc.tensor.matmul(
                                ps_v[i][:],
                                x_t[k][:, m * 128 : (m + 1) * 128],
                                wv_t[k][:],
                                start=(k == 0), stop=(k == N_KT - 1),
                            )
                    for i in range(8):
                        m = wave * 8 + i
                        nc.vector.tensor_tensor(
                            vp[m][:], ps_v[i][:], bv_t[:], mybir.AluOpType.add
                        )

            # weight prefetch for phase D (scheduler fills DMA gaps)
            for k in range(N_KT):
                nc.sync.dma_start(wp_t[k][:], wp_d.ap()[k * 128 : (k + 1) * 128, :])

            # ---- phase B: attention -----------------------------------
            with (
                tc.tile_pool(name="p", bufs=4) as p_pool,
                tc.tile_pool(name="rr", bufs=4) as rr_pool,
                tc.tile_pool(name="rcp", bufs=2) as rcp_pool,
                tc.tile_pool(name="psc", bufs=2, space="PSUM") as ps_sc,
                tc.tile_pool(name="pacc", bufs=1, space="PSUM") as ps_acc,
            ):
                for p in range(2):  # head pair: local heads 2p, 2p+1
                    for cq in range(N_SQ):
                        qsl = slice(cq * SQ_CHUNK, (cq + 1) * SQ_CHUNK)
                        # col-tiled PV pair: head a in array cols 0:63 ->
                        # acc_a rows 0:64, head b in cols 64:127 -> acc_b rows
                        # 64:128 (separate tiles keep the accumulation groups
                        # in distinct psum banks)
                        acc_a = ps_acc.tile([128, SQ_CHUNK], FP32, tag="acca")
                        acc_b = ps_acc.tile([128, SQ_CHUNK], FP32, tag="accb")
                        ra = rr_pool.tile([128, SQ_CHUNK], FP32R, tag="ra")
                        rb2 = rr_pool.tile([128, SQ_CHUNK], FP32R, tag="rb2")
                        for t in range(N_SK):
                            tsl = slice(t * 128, (t + 1) * 128)
                            sca = ps_sc.tile([128, SQ_CHUNK], FP32, tag="sc")
                            scb = ps_sc.tile([128, SQ_CHUNK], FP32, tag="sc")
                            for u in range(SQ_CHUNK // 512):
                                usl = slice(u * 512, (u + 1) * 512)
                                gsl = slice(cq * SQ_CHUNK + u * 512,
                                            cq * SQ_CHUNK + (u + 1) * 512)
                                nc.tensor.matmul(
                                    sca[:, usl],
                                    kT[p][0:64, tsl],
                                    qT[p][0:64, gsl],
                                    start=True, stop=True,
                                    tile_position=(0, 0),
                                )
                                nc.tensor.matmul(
                                    scb[:, usl],
                                    kT[p][64:128, tsl],
                                    qT[p][64:128, gsl],
                                    start=True, stop=True,
                                    tile_position=(64, 0),
                                )
                            pa = p_pool.tile([128, SQ_CHUNK], BF16, tag="pt")
                            pb = p_pool.tile([128, SQ_CHUNK], BF16, tag="pt")
                            nc.scalar.activation(
                                pa[:], sca[:],
                                mybir.ActivationFunctionType.Exp, scale=0.125,
                            )
                            nc.scalar.activation(
                                pb[:], scb[:],
                                mybir.ActivationFunctionType.Exp, scale=0.125,
                            )
                            if t == 0:
                                nc.vector.tensor_copy(ra[:], pa[:])
                                nc.vector.tensor_copy(rb2[:], pb[:])
                            else:
                                nc.vector.tensor_add(ra[:], ra[:], pa[:])
                                nc.vector.tensor_add(rb2[:], rb2[:], pb[:])
                            for u in range(SQ_CHUNK // 512):
                                usl = slice(u * 512, (u + 1) * 512)
                                nc.tensor.matmul(
                                    acc_a[0:64, usl],
                                    vp[t][:, p * 128 : p * 128 + 64],
                                    pa[:, usl],
                                    start=(t == 0), stop=(t == N_SK - 1),
                                    tile_position=(0, 0),
                                )
                                nc.tensor.matmul(
                                    acc_b[64:128, usl],
                                    vp[t][:, p * 128 + 64 : p * 128 + 128],
                                    pb[:, usl],
                                    start=(t == 0), stop=(t == N_SK - 1),
                                    tile_position=(0, 64),
                                )
                        # normalize: reduce rowsums over the 128 sk partitions
                        # (broadcast to all rows by the all-ones stationary)
                        for racc, acc, rows, half in (
                            (ra, acc_a, slice(0, 64), 0),
                            (rb2, acc_b, slice(64, 128), 1),
                        ):
                            rbp = ps_sc.tile([128, SQ_CHUNK], FP32, tag="sc")
                            for u in range(SQ_CHUNK // 512):
                                usl = slice(u * 512, (u + 1) * 512)
                                nc.tensor.matmul(
                                    rbp[:, usl], ones_t[:], racc[:, usl],
                                    start=True, stop=True,
                                )
                            rc = rcp_pool.tile([64, SQ_CHUNK], FP32, tag="rc")
                            nc.vector.reciprocal_approx_fast(rc[:], rbp[rows, :])
                            nc.vector.tensor_tensor(
                                aoT[p][64 * half : 64 * half + 64, qsl],
                                acc[rows, :], rc[:],
                                mybir.AluOpType.mult,
                            )
                        # gather this (pair, sq-chunk) while compute continues
                        nc.sync.dma_start(ag_in[p][cq].ap(), aoT[p][:, qsl])
                        nc.gpsimd.collective_compute(
                            "AllGather",
                            mybir.AluOpType.bypass,
                            replica_groups=groups,
                            ins=[ag_in[p][cq].ap()],
                            outs=[ag_out[p].ap()[cq * 512 : (cq + 1) * 512, :]],
                        )


            # ---- phase D: output projection on own 512 rows -----------
            with (
                tc.tile_pool(name="at", bufs=1) as at_pool,
                tc.tile_pool(name="outp", bufs=4) as out_pool,
                tc.tile_pool(name="psD", bufs=4, space="PSUM") as psD,
            ):
                # logical head-row block k lives in ag_out[k%2]; the
                # gathered rows are stacked [sq-chunk][group-rank][128],
                # and this core's sequence window picks chunk (rank//2)
                # at column offset (rank%2)*512
                pid = nc.gpsimd.partition_id()
                rank = pid % GROUP
                col0 = (rank % 2) * S_OWN
                at_t = [at_pool.tile([128, S_OWN], BF16, name=f"at{k}", tag=f"at{k}")
                        for k in range(N_KT)]
                for k in range(N_KT):
                    row0 = (rank // 2) * 512 + 128 * (k // 2)
                    nc.gpsimd.dma_start(
                        at_t[k][:],
                        ag_out[k % 2].ap()[bass.ds(row0, 128), bass.ds(col0, S_OWN)],
                    )
                for m in range(S_OWN // 128):
                    msl = slice(m * 128, (m + 1) * 128)
                    for nb in range(2):
                        nsl = slice(nb * 512, (nb + 1) * 512)
                        ps = psD.tile([128, 512], FP32, tag="psD")
                        korder = [0, 2, 4, 6, 1, 3, 5, 7]
                        for ki, k in enumerate(korder):
                            nc.tensor.matmul(
                                ps[:],
                                at_t[k][:, msl],
                                wp_t[k][:, nsl],
                                start=(ki == 0),
                                stop=(ki == N_KT - 1),
                            )
                        ot = out_pool.tile([128, 512], FP32, tag="ot")
                        nc.vector.tensor_tensor(
                            ot[:], ps[:], bp_t[:, nsl], mybir.AluOpType.add
                        )
                        nc.sync.dma_start(out_d.ap()[msl, nsl], ot[:])

    nc.compile()
    return nc


def _get_program():
    global _compiled
    if _compiled is None:
        _compiled = _build()
    return _compiled


def _make_in_maps(x, w_qkv, b_qkv, w_proj, b_proj):
    x = np.asarray(x, dtype=np.float32)
    w_qkv = np.asarray(w_qkv, dtype=np.float32)
    b_qkv = np.asarray(b_qkv, dtype=np.float32)
    w_proj = np.asarray(w_proj, dtype=np.float32)
    b_proj = np.asarray(b_proj, dtype=np.float32)

    global _ONES16
    if _ONES16 is None:
        _ONES16 = np.ones((128, 8), dtype=BF16_NP)
    wp16 = w_proj.astype(BF16_NP)
    bp_b = np.ascontiguousarray(np.broadcast_to(b_proj.reshape(1, D), (128, D)))
    in_maps = []
    for c in range(N_CORES):
        g, r = c // GROUP, c % GROUP
        xT = np.ascontiguousarray(x[g].T)
        in_maps.append(
            {
                "xT": xT.astype(BF16_NP),
                "wq": w_qkv[:, 0 * D + r * LHD : 0 * D + (r + 1) * LHD].astype(BF16_NP),
                "wk": w_qkv[:, 1 * D + r * LHD : 1 * D + (r + 1) * LHD].astype(BF16_NP),
                "wv": w_qkv[:, 2 * D + r * LHD : 2 * D + (r + 1) * LHD].astype(BF16_NP),
                "wp": wp16,
                "bq": np.ascontiguousarray(b_qkv[0 * D + r * LHD : 0 * D + (r + 1) * LHD].reshape(LHD, 1)),
                "bk": np.ascontiguousarray(b_qkv[1 * D + r * LHD : 1 * D + (r + 1) * LHD].reshape(LHD, 1)),
                "bv": np.ascontiguousarray(
                    np.broadcast_to(
                        b_qkv[2 * D + r * LHD : 2 * D + (r + 1) * LHD].reshape(1, LHD),
                        (128, LHD),
                    )
                ),
                "bp": bp_b,
                "ones": _ONES,
                "ones16": _ONES16,
            }
        )
    return in_maps


def _assemble(results):
    out = np.empty((B, S, D), dtype=np.float32)
    for c in range(N_CORES):
        g, r = c // GROUP, c % GROUP
        out[g, r * S_OWN : (r + 1) * S_OWN, :] = results[c]["out"]
    return out


def kernel(x, w_qkv, b_qkv, w_proj, b_proj):
    nc = _get_program()
    in_maps = _make_in_maps(x, w_qkv, b_qkv, w_proj, b_proj)
    res = run_bass_kernel_spmd(nc, in_maps, list(range(N_CORES)))
    return _assemble(res.results)


# revision 30
# speedup vs baseline: 1.0578x; 1.0578x over previous
"""Multi-head attention (B=2, S=2048, D=1024, H=16) on 8 Trainium2 NeuronCores.

Sharding: tensor-parallel over heads x data-parallel over batch.
  core c -> batch g = c // 4, head group r = c % 4 (global heads 4r..4r+3).
Each core computes qkv projections for its 4 heads (w_qkv column slices),
attention for those heads over the full sequence of its batch, then an
AllToAll inside each 4-core group converts the head-sharded attention
output into a sequence-sharded one, so every core runs the full output
projection for its own 512 sequence rows. Host-side work is only
slicing/transposition of inputs and concatenation of outputs.

Device algorithm (per core):
  qT, kT   [256, 2048]  (partition = head*64+hd, free = seq)
  V        [2048, 256]  (partition = seq, free = head*64+hd)
  per (sq-chunk 1024, head pair):
    per sk-tile (128 keys):
      scoresT[sk, sq] = kT.T @ qT   (two K=64 matmuls row-packed in PE)
      pT = exp(0.125 * scoresT)     (ACT, PSUM -> SBUF)
      rowsum_acc += pT              (DVE)
      outT += V.T-slice @ pT        (PSUM accumulate, col-packed pairs)
    rowsum broadcast = ones[128,128].T @ rowsum_acc  (reduce over sk partitions)
    outT *= 1/rowsum  (DVE reciprocal + mul) -> attn_outT [256, 2048]
  AllToAll( [4 s-chunks x 256 head-rows, 512] ) -> [1024 head-rows, 512 own s]
  out[512, 1024] = attn_outT.T @ w_proj + b_proj
"""

import os
import sys

import numpy as np

try:
    import ml_dtypes
    BF16_NP = ml_dtypes.bfloat16
except ImportError:  # pragma: no cover
    BF16_NP = None

for _p in ("/opt/trn_rl_repo",):
    if os.path.isdir(_p) and _p not in sys.path:
        sys.path.append(_p)

import concourse.bass as bass  # noqa: E402
import concourse.mybir as mybir  # noqa: E402
import concourse.tile as tile  # noqa: E402
from concourse import bacc  # noqa: E402
from concourse.bass_utils import run_bass_kernel_spmd  # noqa: E402

B, S, D = 2, 2048, 1024
H, HD = 16, 64
N_CORES = 8
GROUP = 4  # cores per batch group
LH = H // GROUP  # local heads per core = 4
LHD = LH * HD  # 256 local head dims
S_OWN = S // GROUP  # 512 sequence rows owned for the projection
FP32 = mybir.dt.float32
FP32R = mybir.dt.float32r
BF16 = mybir.dt.bfloat16

SQ_CHUNK = 1024  # query-range processed per inner attention pass
N_SQ = S // SQ_CHUNK  # 2
N_SK = S // 128  # 16 key tiles
N_KT = D // 128  # 8 contraction tiles for the projections

_compiled = None
_ONES = np.ones((128, 128), dtype=np.float32)
_ONES16 = None  # set lazily (needs ml_dtypes)


def _build():
    nc = bacc.Bacc(
        "TRN2", target_bir_lowering=False, debug=False, num_devices=N_CORES
    )

    xT_d = nc.dram_tensor("xT", [D, S], BF16, kind="ExternalInput")
    wq_d = nc.dram_tensor("wq", [D, LHD], BF16, kind="ExternalInput")
    wk_d = nc.dram_tensor("wk", [D, LHD], BF16, kind="ExternalInput")
    wv_d = nc.dram_tensor("wv", [D, LHD], BF16, kind="ExternalInput")
    wp_d = nc.dram_tensor("wp", [D, D], BF16, kind="ExternalInput")
    bq_d = nc.dram_tensor("bq", [LHD, 1], FP32, kind="ExternalInput")
    bk_d = nc.dram_tensor("bk", [LHD, 1], FP32, kind="ExternalInput")
    bv_d = nc.dram_tensor("bv", [128, LHD], FP32, kind="ExternalInput")
    bp_d = nc.dram_tensor("bp", [128, D], FP32, kind="ExternalInput")
    ones_d = nc.dram_tensor("ones", [128, 128], FP32R, kind="ExternalInput")
    ones16_d = nc.dram_tensor("ones16", [128, 8], BF16, kind="ExternalInput")
    out_d = nc.dram_tensor("out", [S_OWN, D], FP32, kind="ExternalOutput")

    # Two AllGathers (one per head pair) inside each 4-core batch group;
    # the first runs while the second pair's attention still computes.
    ag_in = [[nc.dram_tensor(f"ag_in{p}_{c}", [128, SQ_CHUNK], BF16)
              for c in range(N_SQ)] for p in range(2)]
    ag_out = [nc.dram_tensor(f"ag_out{p}", [N_SQ * GROUP * 128, SQ_CHUNK], BF16)
              for p in range(2)]
    groups = [[0, 1, 2, 3], [4, 5, 6, 7]]

    with tile.TileContext(nc) as tc:
        import contextlib

        with contextlib.ExitStack() as stk:
            # ---- long-lived pools -------------------------------------
            qk_pool = stk.enter_context(tc.tile_pool(name="qk", bufs=1))
            v_pool = stk.enter_context(tc.tile_pool(name="v", bufs=1))
            ao_pool = stk.enter_context(tc.tile_pool(name="ao", bufs=1))
            const_pool = stk.enter_context(tc.tile_pool(name="const", bufs=1))
            wp_pool = stk.enter_context(tc.tile_pool(name="wp", bufs=1))

            qT = [qk_pool.tile([128, S], BF16, name=f"qT{j}", tag=f"qT{j}") for j in range(2)]
            kT = [qk_pool.tile([128, S], BF16, name=f"kT{j}", tag=f"kT{j}") for j in range(2)]
            vp = [v_pool.tile([128, LH * 65], BF16, name=f"v{m}", tag=f"v{m}") for m in range(16)]
            aoT = [ao_pool.tile([128, S], BF16, name=f"ao{p}", tag=f"ao{p}") for p in range(2)]

            ones_t = const_pool.tile([128, 128], FP32R, tag="ones")
            nc.sync.dma_start(ones_t[:], ones_d.ap())
            bq_t = [const_pool.tile([128, 1], FP32, name=f"bq{j}", tag=f"bq{j}")
                    for j in range(2)]
            bk_t = [const_pool.tile([128, 1], FP32, name=f"bk{j}", tag=f"bk{j}")
                    for j in range(2)]
            bv_t = const_pool.tile([128, LHD], FP32, tag="bv")
            bp_t = const_pool.tile([128, D], FP32, tag="bp")
            for j in range(2):
                jsl = slice(j * 128, (j + 1) * 128)
                nc.sync.dma_start(bq_t[j][:], bq_d.ap()[jsl, :])
                nc.sync.dma_start(bk_t[j][:], bk_d.ap()[jsl, :])
            nc.sync.dma_start(bv_t[:], bv_d.ap())
            nc.sync.dma_start(bp_t[:], bp_d.ap())

            wp_t = [wp_pool.tile([128, D], BF16, name=f"wp{k}", tag=f"wp{k}") for k in range(N_KT)]

            # ---- phase A: qkv projections -----------------------------
            with (
                tc.tile_pool(name="x", bufs=1) as x_pool,
                tc.tile_pool(name="w", bufs=1) as w_pool,
                tc.tile_pool(name="psA", bufs=1, space="PSUM") as psA,
            ):
                x_t = [x_pool.tile([128, S], BF16, name=f"x{k}", tag=f"x{k}") for k in range(N_KT)]
                wq_t = [w_pool.tile([128, LHD], BF16, name=f"wq{k}", tag=f"wq{k}") for k in range(N_KT)]
                wk_t = [w_pool.tile([128, LHD], BF16, name=f"wk{k}", tag=f"wk{k}") for k in range(N_KT)]
                wv_t = [w_pool.tile([128, LHD], BF16, name=f"wv{k}", tag=f"wv{k}") for k in range(N_KT)]
                # interleave x/w loads k-major and fan out across four DMA
                # queues so the first contraction tiles land quickly
                dma_engs = [nc.sync, nc.scalar]
                for k in range(N_KT):
                    sl = slice(k * 128, (k + 1) * 128)
                    eng = dma_engs[k % 2]
                    eng.dma_start(x_t[k][:], xT_d.ap()[sl, :])
                    eng2 = dma_engs[(k + 1) % 2]
                    eng2.dma_start(wq_t[k][:], wq_d.ap()[sl, :])
                    eng2.dma_start(wk_t[k][:], wk_d.ap()[sl, :])
                    eng2.dma_start(wv_t[k][:], wv_d.ap()[sl, :])

                # qT / kT / V in PSUM waves, contraction k outermost so the
                # PE follows the xT tiles as they stream in from HBM
                for j in range(2):
                    ps_q = [psA.tile([128, 512], FP32, name=f"psq{j}{sc}", tag=f"psA{sc}") for sc in range(4)]
                    ps_k = [psA.tile([128, 512], FP32, name=f"psk{j}{sc}", tag=f"psA{sc+4}") for sc in range(4)]
                    for k in range(N_KT):
                        for sc in range(4):
                            ssl = slice(sc * 512, (sc + 1) * 512)
                            nc.tensor.matmul(
                                ps_q[sc][:],
                                wq_t[k][:, j * 128 : (j + 1) * 128],
                                x_t[k][:, ssl],
                                start=(k == 0), stop=(k == N_KT - 1),
                            )
                            nc.tensor.matmul(
                                ps_k[sc][:],
                                wk_t[k][:, j * 128 : (j + 1) * 128],
                                x_t[k][:, ssl],
                                start=(k == 0), stop=(k == N_KT - 1),
                            )
                    for sc in range(4):
                        ssl = slice(sc * 512, (sc + 1) * 512)
                        nc.vector.tensor_scalar(
                            qT[j][:, ssl], ps_q[sc][:], bq_t[j][:], None,
                            mybir.AluOpType.add,
                        )
                        nc.vector.tensor_scalar(
                            kT[j][:, ssl], ps_k[sc][:], bk_t[j][:], None,
                            mybir.AluOpType.add,
                        )
                # V: [s-tile 128, 256] = x.T @ wv, two waves of 8 m-tiles
                for wave in range(2):
                    ps_v = [psA.tile([128, LHD], FP32, name=f"psv{wave}{i}", tag=f"psA{i}") for i in range(8)]
                    for k in range(N_KT):
                        for i in range(8):
                            m = wave * 8 + i
                            nc.tensor.matmul(
                                ps_v[i][:],
                                x_t[k][:, m * 128 : (m + 1) * 128],
                                wv_t[k][:],
                                start=(k == 0), stop=(k == N_KT - 1),
                            )
                    for i in range(8):
                        m = wave * 8 + i
                        for h in range(LH):
                            nc.vector.tensor_tensor(
                                vp[m][:, 65 * h : 65 * h + 64],
                                ps_v[i][:, 64 * h : 64 * h + 64],
                                bv_t[:, 64 * h : 64 * h + 64],
                                mybir.AluOpType.add,
                            )
                        nc.sync.dma_start(vp[m][:, 64::65], ones16_d.ap()[:, 0:LH])

            # weight prefetch for phase D (scheduler fills DMA gaps)
            for k in range(N_KT):
                nc.sync.dma_start(wp_t[k][:], wp_d.ap()[k * 128 : (k + 1) * 128, :])

            # ---- phase B: attention -----------------------------------
            with (
                tc.tile_pool(name="p", bufs=4) as p_pool,
                tc.tile_pool(name="rr", bufs=4) as rr_pool,
                tc.tile_pool(name="rcp", bufs=2) as rcp_pool,
                tc.tile_pool(name="psc", bufs=2, space="PSUM") as ps_sc,
                tc.tile_pool(name="pacc", bufs=1, space="PSUM") as ps_acc,
            ):
                for p in range(2):  # head pair: local heads 2p, 2p+1
                    for cq in range(N_SQ):
                        qsl = slice(cq * SQ_CHUNK, (cq + 1) * SQ_CHUNK)
                        # row 64 of each acc collects the softmax denominator
                        # via the ones column appended to V
                        acc_a = ps_acc.tile([65, SQ_CHUNK], FP32, tag="acca")
                        acc_b = ps_acc.tile([65, SQ_CHUNK], FP32, tag="accb")
                        for t in range(N_SK):
                            tsl = slice(t * 128, (t + 1) * 128)
                            sca = ps_sc.tile([128, SQ_CHUNK], FP32, tag="sc")
                            scb = ps_sc.tile([128, SQ_CHUNK], FP32, tag="sc")
                            for u in range(SQ_CHUNK // 512):
                                usl = slice(u * 512, (u + 1) * 512)
                                gsl = slice(cq * SQ_CHUNK + u * 512,
                                            cq * SQ_CHUNK + (u + 1) * 512)
                                nc.tensor.matmul(
                                    sca[:, usl],
                                    kT[p][0:64, tsl],
                                    qT[p][0:64, gsl],
                                    start=True, stop=True,
                                    tile_position=(0, 0),
                                )
                                nc.tensor.matmul(
                                    scb[:, usl],
                                    kT[p][64:128, tsl],
                                    qT[p][64:128, gsl],
                                    start=True, stop=True,
                                    tile_position=(64, 0),
                                )
                            pa = p_pool.tile([128, SQ_CHUNK], BF16, tag="pt")
                            pb = p_pool.tile([128, SQ_CHUNK], BF16, tag="pt")
                            nc.scalar.activation(
                                pa[:], sca[:],
                                mybir.ActivationFunctionType.Exp, scale=0.125,
                            )
                            nc.scalar.activation(
                                pb[:], scb[:],
                                mybir.ActivationFunctionType.Exp, scale=0.125,
                            )
                            for u in range(SQ_CHUNK // 512):
                                usl = slice(u * 512, (u + 1) * 512)
                                nc.tensor.matmul(
                                    acc_a[:, usl],
                                    vp[t][:, 65 * (2 * p) : 65 * (2 * p) + 65],
                                    pa[:, usl],
                                    start=(t == 0), stop=(t == N_SK - 1),
                                )
                                nc.tensor.matmul(
                                    acc_b[:, usl],
                                    vp[t][:, 65 * (2 * p + 1) : 65 * (2 * p + 1) + 65],
                                    pb[:, usl],
                                    start=(t == 0), stop=(t == N_SK - 1),
                                )
                        # normalize: 1/rowsum broadcast across the 64 head dims
                        for acc, half in ((acc_a, 0), (acc_b, 1)):
                            rrow = rr_pool.tile([1, SQ_CHUNK], FP32R, tag="rrow")
                            nc.vector.tensor_copy(rrow[:], acc[64:65, :])
                            rb = ps_sc.tile([64, SQ_CHUNK], FP32, tag="sc")
                            for u in range(SQ_CHUNK // 512):
                                usl = slice(u * 512, (u + 1) * 512)
                                nc.tensor.matmul(
                                    rb[:, usl], ones_t[0:1, 0:64], rrow[:, usl],
                                    start=True, stop=True,
                                )
                            rc = rcp_pool.tile([64, SQ_CHUNK], FP32, tag="rc")
                            nc.vector.reciprocal_approx_fast(rc[:], rb[:])
                            nc.vector.tensor_tensor(
                                aoT[p][64 * half : 64 * half + 64, qsl],
                                acc[0:64, :], rc[:],
                                mybir.AluOpType.mult,
                            )
                        # gather this (pair, sq-chunk) while compute continues
                        nc.sync.dma_start(ag_in[p][cq].ap(), aoT[p][:, qsl])
                        nc.gpsimd.collective_compute(
                            "AllGather",
                            mybir.AluOpType.bypass,
                            replica_groups=groups,
                            ins=[ag_in[p][cq].ap()],
                            outs=[ag_out[p].ap()[cq * 512 : (cq + 1) * 512, :]],
                        )


            # ---- phase D: output projection on own 512 rows -----------
            with (
                tc.tile_pool(name="at", bufs=1) as at_pool,
                tc.tile_pool(name="outp", bufs=4) as out_pool,
                tc.tile_pool(name="psD", bufs=4, space="PSUM") as psD,
            ):
                # logical head-row block k lives in ag_out[k%2]; the
                # gathered rows are stacked [sq-chunk][group-rank][128],
                # and this core's sequence window picks chunk (rank//2)
                # at column offset (rank%2)*512
                pid = nc.gpsimd.partition_id()
                rank = pid % GROUP
                col0 = (rank % 2) * S_OWN
                at_t = [at_pool.tile([128, S_OWN], BF16, name=f"at{k}", tag=f"at{k}")
                        for k in range(N_KT)]
                for k in range(N_KT):
                    row0 = (rank // 2) * 512 + 128 * (k // 2)
                    nc.gpsimd.dma_start(
                        at_t[k][:],
                        ag_out[k % 2].ap()[bass.ds(row0, 128), bass.ds(col0, S_OWN)],
                    )
                for m in range(S_OWN // 128):
                    msl = slice(m * 128, (m + 1) * 128)
                    for nb in range(2):
                        nsl = slice(nb * 512, (nb + 1) * 512)
                        ps = psD.tile([128, 512], FP32, tag="psD")
                        korder = [0, 2, 4, 6, 1, 3, 5, 7]
                        for ki, k in enumerate(korder):
                            nc.tensor.matmul(
                                ps[:],
                                at_t[k][:, msl],
                                wp_t[k][:, nsl],
                                start=(ki == 0),
                                stop=(ki == N_KT - 1),
                            )
                        ot = out_pool.tile([128, 512], FP32, tag="ot")
                        nc.vector.tensor_tensor(
                            ot[:], ps[:], bp_t[:, nsl], mybir.AluOpType.add
                        )
                        nc.sync.dma_start(out_d.ap()[msl, nsl], ot[:])

    nc.compile()
    return nc


def _get_program():
    global _compiled
    if _compiled is None:
        _compiled = _build()
    return _compiled


def _make_in_maps(x, w_qkv, b_qkv, w_proj, b_proj):
    x = np.asarray(x, dtype=np.float32)
    w_qkv = np.asarray(w_qkv, dtype=np.float32)
    b_qkv = np.asarray(b_qkv, dtype=np.float32)
    w_proj = np.asarray(w_proj, dtype=np.float32)
    b_proj = np.asarray(b_proj, dtype=np.float32)

    global _ONES16
    if _ONES16 is None:
        _ONES16 = np.ones((128, 8), dtype=BF16_NP)
    wp16 = w_proj.astype(BF16_NP)
    bp_b = np.ascontiguousarray(np.broadcast_to(b_proj.reshape(1, D), (128, D)))
    in_maps = []
    for c in range(N_CORES):
        g, r = c // GROUP, c % GROUP
        xT = np.ascontiguousarray(x[g].T)
        in_maps.append(
            {
                "xT": xT.astype(BF16_NP),
                "wq": w_qkv[:, 0 * D + r * LHD : 0 * D + (r + 1) * LHD].astype(BF16_NP),
                "wk": w_qkv[:, 1 * D + r * LHD : 1 * D + (r + 1) * LHD].astype(BF16_NP),
                "wv": w_qkv[:, 2 * D + r * LHD : 2 * D + (r + 1) * LHD].astype(BF16_NP),
                "wp": wp16,
                "bq": np.ascontiguousarray(b_qkv[0 * D + r * LHD : 0 * D + (r + 1) * LHD].reshape(LHD, 1)),
                "bk": np.ascontiguousarray(b_qkv[1 * D + r * LHD : 1 * D + (r + 1) * LHD].reshape(LHD, 1)),
                "bv": np.ascontiguousarray(
                    np.broadcast_to(
                        b_qkv[2 * D + r * LHD : 2 * D + (r + 1) * LHD].reshape(1, LHD),
                        (128, LHD),
                    )
                ),
                "bp": bp_b,
                "ones": _ONES,
                "ones16": _ONES16,
            }
        )
    return in_maps


def _assemble(results):
    out = np.empty((B, S, D), dtype=np.float32)
    for c in range(N_CORES):
        g, r = c // GROUP, c % GROUP
        out[g, r * S_OWN : (r + 1) * S_OWN, :] = results[c]["out"]
    return out


def kernel(x, w_qkv, b_qkv, w_proj, b_proj):
    nc = _get_program()
    in_maps = _make_in_maps(x, w_qkv, b_qkv, w_proj, b_proj)
    res = run_bass_kernel_spmd(nc, in_maps, list(range(N_CORES)))
    return _assemble(res.results)
